# revision 42
# baseline (speedup 1.0000x reference)
"""Multi-head causal attention with RoPE on 8 Trainium2 NeuronCores.

Problem: x[2, 2048, 1024], 16 heads, d_k=64, RoPE(theta=1e4), causal,
weights W{q,k,v,o}[1024, 1024] stored [d_out, d_in].

Sharding: 2 batches x 4 head-groups -> 8 cores. Core c handles batch
c//4, heads 4*(c%4) .. 4*(c%4)+4. Each core computes its 4 heads'
attention plus the partial o_proj for its head columns; the host sums
the 4 partials per batch (the "all-reduce after o_proj").

v2 design notes (vs the f32r v1):
- ALL matmul operands are bf16 (weights, x, Q/K, mask, ex, V, ao).
  fp32-family matmuls serialize a ~210ns LDWEIGHTS with each MATMUL
  (no background weight buffer) and block row-group concurrency; bf16
  gets FWL + the bg buffer, so MMs stream at ~N/2.4GHz.
- Q/K live in a permuted [e', s] parity layout, e' = parity*128 +
  h*32 + j, so RoPE is full-width DVE/GPSIMD ops and the score MMs
  contract head h over partition rows [32h, 32h+32) of both parity
  tiles (4 heads concurrent via row tile_position).
- Scores are key-major (scoresT [k, q]); exp is batched 2 heads per
  ACT instruction ([128, 2, SC] PSUM group tiles, tags A/B so exp(kt)
  overlaps scores(kt) of the other group).
- attn@V packs 2 heads per PSUM bank via col tile_position (M=64);
  the softmax denominator accumulates in a separate PSUM bank via
  M=1 matmuls (ones lhsT), lagged one kt behind so the bank's tag
  reuse by the rb broadcast never stalls the PE queue.
- 1/den = exp(-ln(den)) on ACT (ln+exp share an ACT table set;
  Reciprocal would force a ~2.7us table switch per use). The per-head
  recip rows are broadcast to [128, SC] by K=1 matmuls (sel lhsT).
- o_proj for q-chunk qc is emitted interleaved into qc+1's kt loop
  (dedicated PSUM bank) so the out DMA overlaps attention.
- Host pre-permutes every DRAM tensor so each DMA is a contiguous
  per-partition pattern (the v1 8-way interleaved x gather took >10us
  just to issue).
"""

import sys

if "/opt/trn_rl_repo" not in sys.path:
    sys.path.insert(0, "/opt/trn_rl_repo")

import numpy as np

import concourse.bass as bass
import concourse.mybir as mybir
import concourse.tile as tile
from concourse import bacc, library_config
from concourse.bass_utils import run_bass_kernel_spmd

F32 = mybir.dt.float32
BF16 = mybir.dt.bfloat16
EXP = mybir.ActivationFunctionType.Exp
LN = mybir.ActivationFunctionType.Ln

B = 2
S = 2048
D = 1024
H = 16
DK = 64
HC = 4          # heads per core
E = HC * DK     # 256 d_out columns per core
THETA = 10000.0
SC = 512        # seq chunk (psum free dim)
NSC = S // SC   # 4
NST = S // 128  # 16 s-tiles
NEG = -1.0e30

_COMPILED = None

# bisect toggles for HW-risky features
EXP_GROUPED = True    # one ACT exp over a 2-bank PSUM group tile
GPSIMD_ROPE = True    # RoPE combines on GPSIMD instead of DVE
SCHRAUD = False       # offload some exp groups to DVE (Schraudolph bf16);
                      # measured: costs DVE ~2x what it saves ACT, and PE
                      # (not ACT) is the pacer
# Schraudolph constants: bf16 bits of exp(s) ~= s*128*log2(e) + 127*128,
# with a small minimax bias correction.
SCH_MUL = 184.6635
SCH_ADD = 16250.5


def _build():
    nc = bacc.Bacc("TRN2", target_bir_lowering=False, debug=False, num_devices=8)

    xb = nc.dram_tensor("xb", [128, NSC, 8, SC], BF16, kind="ExternalInput")
    wq = nc.dram_tensor("wq", [128, 8, E], BF16, kind="ExternalInput")
    wk = nc.dram_tensor("wk", [128, 8, E], BF16, kind="ExternalInput")
    wv = nc.dram_tensor("wv", [128, 8, E], BF16, kind="ExternalInput")
    wo = nc.dram_tensor("wo", [128, 2, D], BF16, kind="ExternalInput")
    cs = nc.dram_tensor("cs", [128, NSC, 2, SC], F32, kind="ExternalInput")
    misc = nc.dram_tensor("misc", [128, 513], BF16, kind="ExternalInput")
    out_d = nc.dram_tensor("out", [S, D], BF16, kind="ExternalOutput")

    with tile.TileContext(nc) as tc:
        with (
            tc.tile_pool(name="const", bufs=1) as const,
            tc.tile_pool(name="persist", bufs=1) as persist,
            tc.tile_pool(name="xp", bufs=2) as xp,
            tc.tile_pool(name="cspool", bufs=2) as cspool,
            tc.tile_pool(name="ropet", bufs=2) as ropet,
            tc.tile_pool(name="expool", bufs=4) as expool,
            tc.tile_pool(name="schpool", bufs=2) as schpool,
            tc.tile_pool(name="rpool", bufs=2) as rpool,
            tc.tile_pool(name="sopool", bufs=3) as sopool,
        ):
            # ---- persistent activations -----------------------------
            q0_sb = persist.tile([128, S], BF16)   # parity-0 rotated Q
            q1_sb = persist.tile([128, S], BF16)
            k0_sb = persist.tile([128, S], BF16)
            k1_sb = persist.tile([128, S], BF16)
            v_sb = persist.tile([128, NST, E], BF16)   # [k, s_tile, h*64+dk]
            ao_sb = persist.tile([128, 2, S], BF16)    # o_proj lhsT, pair-major

            # ---- early constant + first-chunk loads -----------------
            wq_sb = const.tile([128, 8, E], BF16)
            wk_sb = const.tile([128, 8, E], BF16)
            wv_sb = const.tile([128, 8, E], BF16)

            x_tiles = [None] * NSC
            cs_tiles = [None] * NSC

            def fetch_chunk(c):
                x_tiles[c] = xp.tile([128, 8, SC], BF16, name=f"x_{c}", tag="x")
                nc.sync.dma_start(x_tiles[c][:], xb[:, c])
                cs_tiles[c] = cspool.tile([128, 2, SC], F32,
                                          name=f"cs_{c}", tag="cs")
                nc.sync.dma_start(cs_tiles[c][:], cs[:, c])

            # DMA queue order: x0, wq, wk, cs0, wv, then per-chunk
            # prefetches, then the late consts (wo/mask/misc).
            # first chunk: split the x DMA so the dc 0-3 matmuls can
            # start while dc 4-7 is still in flight
            x_tiles[0] = xp.tile([128, 8, SC], BF16, name="x_0", tag="x")
            nc.sync.dma_start(x_tiles[0][:, 0:4, :], xb[:, 0, 0:4])
            nc.sync.dma_start(wq_sb[:], wq[:])
            nc.sync.dma_start(x_tiles[0][:, 4:8, :], xb[:, 0, 4:8])
            nc.sync.dma_start(wk_sb[:], wk[:])
            cs_tiles[0] = cspool.tile([128, 2, SC], F32, name="cs_0", tag="cs")
            nc.sync.dma_start(cs_tiles[0][:], cs[:, 0])
            nc.sync.dma_start(wv_sb[:], wv[:])

            # ---- stage 1: QKV projections + RoPE + V layout ---------
            with tc.tile_pool(name="ps1", bufs=1, space="PSUM") as ps1:
                for c in range(NSC):
                    if c + 1 < NSC:
                        fetch_chunk(c + 1)
                    sl = slice(SC * c, SC * (c + 1))
                    x_sb = x_tiles[c]

                    pq = ps1.tile([128, 2, SC], F32, name=f"pq_{c}",
                                  tag=f"pq{c % 2}")
                    pk = ps1.tile([128, 2, SC], F32, name=f"pk_{c}", tag="pk")
                    for t in range(2):
                        es = slice(128 * t, 128 * (t + 1))
                        for dc in range(8):
                            nc.tensor.matmul(
                                pq[:, t, :], wq_sb[:, dc, es], x_sb[:, dc, :],
                                start=(dc == 0), stop=(dc == 7))
                    for t in range(2):
                        es = slice(128 * t, 128 * (t + 1))
                        for dc in range(8):
                            nc.tensor.matmul(
                                pk[:, t, :], wk_sb[:, dc, es], x_sb[:, dc, :],
                                start=(dc == 0), stop=(dc == 7))
                    pv = [ps1.tile([128, 2, E], F32, name=f"pv{t}_{c}",
                                   tag=f"pv{t}") for t in range(2)]
                    for st in range(4):
                        ssl = slice(128 * st, 128 * (st + 1))
                        for dc in range(8):
                            nc.tensor.matmul(
                                pv[st // 2][:, st % 2, :],
                                x_sb[:, dc, ssl], wv_sb[:, dc, :],
                                start=(dc == 0), stop=(dc == 7))

                    # RoPE: x1' = x1 c - x2 s ; x2' = x1 s + x2 c
                    # muls on DVE (PSUM reads), combines on GPSIMD (SBUF).
                    C = cs_tiles[c][:, 0, :]
                    Sn = cs_tiles[c][:, 1, :]
                    for name, pp, d0, d1 in (
                        ("q", pq, q0_sb, q1_sb),
                        ("k", pk, k0_sb, k1_sb),
                    ):
                        t0 = ropet.tile([128, SC], F32, name=f"t0{name}{c}",
                                        tag="ta")
                        t1 = ropet.tile([128, SC], F32, name=f"t1{name}{c}",
                                        tag="tb")
                        t2 = ropet.tile([128, SC], F32, name=f"t2{name}{c}",
                                        tag="tc")
                        t3 = ropet.tile([128, SC], F32, name=f"t3{name}{c}",
                                        tag="td")
                        eng = nc.gpsimd if GPSIMD_ROPE else nc.vector
                        nc.vector.tensor_mul(t0[:], pp[:, 0, :], C)
                        nc.vector.tensor_mul(t1[:], pp[:, 1, :], Sn)
                        eng.tensor_sub(d0[:, sl], t0[:], t1[:])
                        nc.vector.tensor_mul(t2[:], pp[:, 0, :], Sn)
                        nc.vector.tensor_mul(t3[:], pp[:, 1, :], C)
                        eng.tensor_add(d1[:, sl], t2[:], t3[:])

                    # V into [k, h*64+dk] layout
                    for st in range(4):
                        nc.scalar.copy(
                            v_sb[:, 4 * c + st, :],
                            pv[st // 2][:, st % 2, :])

            # ---- late consts (queue behind stage-1 DMAs) ------------
            wo_sb = const.tile([128, 2, D], BF16)
            nc.sync.dma_start(wo_sb[:], wo[:])
            misc_sb = const.tile([128, 513], BF16)
            nc.sync.dma_start(misc_sb[:], misc[:])
            # trimask[k, g, qq] = 0 where qq < k (upper triangle of the
            # diagonal 128-col block), duplicated for the 2-head group
            tri2 = misc_sb[:, 0:256].rearrange("p (b c) -> p b c", c=128)
            onesK = misc_sb[:, 256:257]
            sel = [misc_sb[:, 257:385], misc_sb[:, 385:513]]

            # ---- stage 2: attention + interleaved o_proj ------------
            with tc.tile_pool(name="ps2", bufs=1, space="PSUM") as ps2:
                s3_queue = []

                def s3_unit(qc, st, dc, tag="po", act_copy=False):
                    """One o_proj piece: po = ao[:, :, ssl].T @ wo[:, :, dsl]."""
                    stg = 4 * qc + st
                    ssl = slice(128 * stg, 128 * (stg + 1))
                    dsl = slice(512 * dc, 512 * (dc + 1))
                    po = ps2.tile([128, 512], F32, name=f"po_{stg}_{dc}",
                                  tag=tag)
                    for pr in range(2):
                        nc.tensor.matmul(
                            po[:], ao_sb[:, pr, ssl], wo_sb[:, pr, dsl],
                            start=(pr == 0), stop=(pr == 1))
                    so = s3_unit.so
                    if dc == 0:
                        so = sopool.tile([128, 2, 512], BF16,
                                         name=f"so_{stg}", tag="so")
                        s3_unit.so = so
                    if act_copy:
                        nc.scalar.copy(so[:, dc, :], po[:])
                    else:
                        nc.vector.tensor_copy(so[:, dc, :], po[:])
                    if dc == 1:
                        nc.sync.dma_start(out_d[ssl, :], so[:])

                s3_unit.so = None

                for qc in range(NSC):
                    qsl = slice(SC * qc, SC * (qc + 1))
                    nkt = 4 * qc + 4
                    # av/den banks hold interleaved accumulation regions
                    # (2 heads / 4 rows per bank), which HW has_written and
                    # the sim's zero-region model both track per-bank. So:
                    # memset to 0, then accumulate-only matmuls
                    # (start=False): correct regardless of stale bits.
                    av = [ps2.tile([128, SC], F32, name=f"av{g}_{qc}",
                                   tag=f"av{g}") for g in range(2)]
                    den = ps2.tile([128, SC], F32, name=f"den_{qc}", tag="den")
                    nc.vector.memset(av[0][:], 0.0)
                    nc.vector.memset(av[1][:], 0.0)
                    # unused den rows hold 1.0 so ln() stays finite
                    nc.vector.memset(den[:], 1.0)
                    for h in range(HC):
                        nc.vector.memset(den[32 * h:32 * h + 1, :], 0.0)

                    prev = None   # (ex tile, kt, w) awaiting AV
                    pden = None   # (ex tile, kt, w) awaiting denominator
                    for kt in range(nkt):
                        ksl = slice(128 * kt, 128 * (kt + 1))
                        diag = kt >= 4 * qc
                        m = kt - 4 * qc
                        w = 128 * m if diag else 0

                        # par-major waves: all 4 heads' row groups run
                        # concurrently within each parity wave
                        scs = [ps2.tile([128, 2, SC], F32,
                                        name=f"sc{g}_{qc}_{kt}",
                                        tag=f"sc{g}") for g in range(2)]
                        for par, ksb, qsb in ((0, k0_sb, q0_sb),
                                              (1, k1_sb, q1_sb)):
                            for g in range(2):
                                for hl in range(2):
                                    h = 2 * g + hl
                                    hp = slice(32 * h, 32 * (h + 1))
                                    nc.tensor.matmul(
                                        scs[g][:, hl, w:SC], ksb[hp, ksl],
                                        qsb[hp, qsl][:, w:SC],
                                        start=(par == 0),
                                        stop=(par == 1),
                                        tile_position=(96, 0) if h == 3
                                        else None)
                        exs = []
                        for g in range(2):
                            ex = expool.tile([128, 2, SC], BF16,
                                             name=f"ex{g}_{qc}_{kt}",
                                             tag=f"ex{g}")
                            if SCHRAUD and not diag and kt % 2 == 0 and g == 1:
                                # Schraudolph on DVE: bf16 bits of exp(s)
                                tt = schpool.tile([128, 2, SC], F32,
                                                  name=f"tt_{qc}_{kt}",
                                                  tag="sch")
                                nc.vector.tensor_scalar(
                                    tt[:, :, w:SC], scs[g][:, :, w:SC],
                                    SCH_MUL, SCH_ADD,
                                    mybir.AluOpType.mult,
                                    mybir.AluOpType.add)
                                nc.vector.tensor_copy(
                                    ex[:, :, w:SC].bitcast(mybir.dt.int16),
                                    tt[:, :, w:SC])
                            elif EXP_GROUPED:
                                nc.scalar.activation(ex[:, :, w:SC],
                                                     scs[g][:, :, w:SC], EXP)
                            else:
                                for hl in range(2):
                                    nc.scalar.activation(
                                        ex[:, hl, w:SC],
                                        scs[g][:, hl, w:SC], EXP)
                            # zero the masked triangle (cols [w, w+128))
                            # on the otherwise-idle GPSIMD engine
                            if diag:
                                nc.gpsimd.tensor_mul(
                                    ex[:, :, w:w + 128],
                                    ex[:, :, w:w + 128], tri2)
                            exs.append(ex)

                        # attn@V for the previous kt (software pipeline)
                        if prev is not None:
                            pex, pkt, pw = prev
                            for g in range(2):
                                for hl in range(2):
                                    h = 2 * g + hl
                                    nc.tensor.matmul(
                                        av[g][64 * hl:64 * hl + 64, pw:SC],
                                        v_sb[:, pkt, 64 * h:64 * h + 64],
                                        pex[g][:, hl, pw:SC],
                                        start=False, stop=False,
                                        skip_group_check=True,
                                        tile_position=(0, 64 * hl))
                        # denominator, lagged one more kt behind
                        if pden is not None:
                            dex, dkt, dw = pden
                            for h in range(HC):
                                nc.tensor.matmul(
                                    den[32 * h:32 * h + 1, dw:SC], onesK,
                                    dex[h // 2][:, h % 2, dw:SC],
                                    start=False, stop=False,
                                    skip_group_check=True,
                                    tile_position=(0, 32 * h))
                        pden = prev
                        prev = (exs, kt, w)

                        # drip-feed the previous q-chunk's deferred work.
                        # rb units (queue head) MUST be emitted at kt==1,
                        # before this q-chunk's first den matmul (kt==2)
                        # re-tags the shared den bank.
                        if kt >= 1:
                            for _ in range(2):
                                if s3_queue:
                                    s3_queue.pop(0)()

                    # drain the software pipeline: den drains FIRST so the
                    # reciprocal (which gates the rb broadcast chain) is
                    # unblocked ~4 waves earlier at every qc boundary
                    pex, pkt, pw = prev
                    for ex_kt_w in (pden, prev):
                        dex, dkt, dw = ex_kt_w
                        for h in range(HC):
                            nc.tensor.matmul(
                                den[32 * h:32 * h + 1, dw:SC], onesK,
                                dex[h // 2][:, h % 2, dw:SC],
                                start=False, stop=False,
                                skip_group_check=True,
                                tile_position=(0, 32 * h))
                    for g in range(2):
                        for hl in range(2):
                            h = 2 * g + hl
                            nc.tensor.matmul(
                                av[g][64 * hl:64 * hl + 64, pw:SC],
                                v_sb[:, pkt, 64 * h:64 * h + 64],
                                pex[g][:, hl, pw:SC],
                                start=False, stop=False,
                                skip_group_check=True,
                                tile_position=(0, 64 * hl))

                    # unnormalized out -> ao (frees av banks), then
                    # 1/den on DVE, broadcast via a full-K sel matmul
                    # (K=1 bf16 matmuls and ACT Ln both crash HW). The
                    # rb matmuls are deferred into the next q-chunk's kt
                    # loop so the in-order PE queue never waits on the
                    # reciprocal.
                    for g in range(2):
                        nc.vector.tensor_copy(ao_sb[:, g, qsl], av[g][:])
                    rf = rpool.tile([128, SC], F32, name=f"rf_{qc}", tag="rf")
                    nc.vector.reciprocal_approx_fast(rf[:], den[:])
                    # DVE, not GPSIMD: this cast gates the rb broadcast and
                    # GPSIMD is ~3x slower per op (measured 1.9us vs 0.7us)
                    rd = rpool.tile([128, SC], BF16, name=f"rd_{qc}", tag="rd")
                    nc.vector.tensor_copy(rd[:], rf[:])

                    def rb_unit(g, qc=qc, rd=rd, qsl=qsl, tag="den"):
                        rb = ps2.tile([128, SC], F32, name=f"rb{g}_{qc}",
                                      tag=tag)
                        nc.tensor.matmul(rb[:], sel[g], rd[:],
                                         start=True, stop=True)
                        nc.vector.tensor_mul(ao_sb[:, g, qsl],
                                             ao_sb[:, g, qsl], rb[:])

                    # rb units first, then this q-chunk's o_proj units
                    s3_queue.append(lambda f=rb_unit, tag="den", **kw:
                                    f(0, tag=tag))
                    s3_queue.append(lambda f=rb_unit, tag="den", **kw:
                                    f(1, tag=tag))
                    for st in range(4):
                        for dc in range(2):
                            s3_queue.append(
                                lambda qc=qc, st=st, dc=dc, **kw:
                                s3_unit(qc, st, dc, **kw))

                # final o_proj drain: everything is idle now, so cycle the
                # tiles through freed banks and alternate the evacuation
                # copies between the idle ACT and DVE
                tags = ["den", "po", "sc0", "sc1", "av0", "av1"]
                i = 0
                while s3_queue:
                    s3_queue.pop(0)(tag=tags[i % len(tags)])
                    i += 1

    nc.compile()
    return nc


def _host_inputs(x, Wq, Wk, Wv, Wo, token_positions):
    """Build the 8 per-core input maps (all host-side numpy prep)."""
    import ml_dtypes
    bf16 = ml_dtypes.bfloat16

    x = np.asarray(x, dtype=np.float32)
    Wq = np.asarray(Wq, dtype=np.float32)
    Wk = np.asarray(Wk, dtype=np.float32)
    Wv = np.asarray(Wv, dtype=np.float32)
    Wo = np.asarray(Wo, dtype=np.float32)
    pos = np.asarray(token_positions, dtype=np.int64)

    # RoPE tables per batch: row h*32+j -> cos/sin(pos[s] * freq[j])
    j = np.arange(0, DK, 2, dtype=np.float64) / DK
    freq = 1.0 / (THETA ** j)                       # [32]
    ang = pos[:, None, :] * freq[None, :, None]     # [B, 32, S]
    cos_b = np.tile(np.cos(ang), (1, 4, 1)).astype(np.float32)  # [B, 128, S]
    sin_b = np.tile(np.sin(ang), (1, 4, 1)).astype(np.float32)
    # cs[b]: [128, NSC, 2, SC]
    cs_b = np.stack([cos_b.reshape(B, 128, NSC, SC),
                     sin_b.reshape(B, 128, NSC, SC)], axis=3)

    # misc: [:, 0:256] causal triangle keep-mask x2 (tri[k, qq] = 1 iff
    # qq >= k), [:, 256] ones, then the two recip-broadcast sel blocks.
    # sel_pr[p, m] = 1 iff p = 32*(2pr + m//64), so sel_pr.T @ rd
    # broadcasts head (2pr + m//64)'s recip row.
    misc_np = np.zeros((128, 513), dtype=bf16)
    kk = np.arange(128)[:, None]
    qq = np.arange(128)[None, :]
    tri = (qq >= kk).astype(bf16)
    misc_np[:, 0:128] = tri
    misc_np[:, 128:256] = tri
    misc_np[:, 256] = 1.0
    for pr in range(2):
        base = 257 + 128 * pr
        misc_np[32 * (2 * pr + 0), base:base + 64] = 1.0
        misc_np[32 * (2 * pr + 1), base + 64:base + 128] = 1.0

    # RoPE-friendly permutation of Wq/Wk rows within each core's slice:
    # e' = parity*128 + h*32 + j  <-  head h, component 2j+parity
    perm = np.empty(E, dtype=np.int64)
    for p in range(2):
        for h in range(HC):
            for jj in range(32):
                perm[p * 128 + h * 32 + jj] = h * DK + 2 * jj + p

    def wlayout(w):   # [d_in=1024, e] -> [128, 8, e]
        return np.ascontiguousarray(
            w.reshape(8, 128, w.shape[1]).transpose(1, 0, 2).astype(bf16))

    in_maps = []
    for core in range(8):
        b, g = core // 4, core % 4
        rows = slice(E * g, E * (g + 1))
        wq_c = (Wq[rows][perm] * (1.0 / np.sqrt(DK))).T  # [1024, 256]
        wk_c = Wk[rows][perm].T
        wv_c = Wv[rows].T
        wo_c = Wo[:, rows].T                             # [256, 1024]
        xb_c = np.ascontiguousarray(
            x[b].T.reshape(8, 128, NSC, SC).transpose(1, 2, 0, 3)
            .astype(bf16))
        in_maps.append({
            "xb": xb_c,
            "wq": wlayout(wq_c),
            "wk": wlayout(wk_c),
            "wv": wlayout(wv_c),
            "wo": np.ascontiguousarray(
                wo_c.reshape(2, 128, D).transpose(1, 0, 2).astype(bf16)),
            "cs": np.ascontiguousarray(cs_b[b]),
            "misc": misc_np,
        })
    return in_maps


def _run(in_maps, trace=False, trace_kwargs=None):
    global _COMPILED
    if _COMPILED is None:
        _COMPILED = _build()
    return run_bass_kernel_spmd(
        _COMPILED, in_maps, list(range(8)), trace=trace,
        **(trace_kwargs or {}))


def _gather(results):
    out = np.empty((B, S, D), dtype=np.float32)
    for b in range(B):
        acc = results[4 * b]["out"].astype(np.float32)
        for g in range(1, 4):
            acc = acc + results[4 * b + g]["out"].astype(np.float32)
        out[b] = acc
    return out


def kernel(x, Wq, Wk, Wv, Wo, token_positions):
    res = _run(_host_inputs(x, Wq, Wk, Wv, Wo, token_positions))
    return _gather(res.results)


def bench(x, Wq, Wk, Wv, Wo, token_positions):
    """Like kernel() but profiles on HW; returns (out, exec_time_ns)."""
    import types

    try:  # register the NTFF hook if the image's antenv lacks it
        from antenv import axon_hooks  # noqa: F401
    except ImportError:
        m = types.ModuleType("antenv.axon_hooks")
        from trn_agent_boot.trn_boot import _ntff_profile_via_ctypes
        hook = _ntff_profile_via_ctypes("/opt/axon/libaxon_pjrt.so")
        m.get_axon_ntff_profile_hook = lambda: hook
        m.set_axon_ntff_profile_hook = lambda h: None
        sys.modules["antenv.axon_hooks"] = m
        import antenv
        antenv.axon_hooks = m

    res = _run(_host_inputs(x, Wq, Wk, Wv, Wo, token_positions), trace=True)
    return _gather(res.results), res.exec_time_ns
